# revision 11
# baseline (speedup 1.0000x reference)
"""Trainium2 Bass kernel for nn_DiscAdvLossForTarget_min.

Math (per batch row, x = logits[0:1000], e = extra logit x[1000]):
    prob_i = softmax(x)_i                  = exp(x_i - e) / sum_j exp(x_j - e)
    log pc_i = log sigmoid(e - x_i)        = -log1p(exp(x_i - e))
    loss = -(1/B) * sum_b sum_i prob_i * log(pc_i)
         = +(1/B) * sum_b U_b / S_b
    where a_i = exp(x_i - e), U_b = sum_i a_i * log1p(a_i), S_b = sum_i a_i.

Device mapping (per core, data-parallel over batch, 8192 rows per core,
64 row-blocks of 128).  HW-measured per-block costs (128x1000, bf16
intermediates): ACT batched act ~835ns/block + ~200-300ns/instr, ACT
per-block Exp w/ bias + accum read 1312ns, DVE tensor_scalar fold
(a = t*exp(-e), accum_out -> S) 1185ns (the accum variant is 1x on
silicon), DVE scalar_tensor_tensor (U product w/ accum) 1119ns.  The
GpSimd/Pool engine cannot do free-axis reductions or accumulation on
TRN2, so the work is balanced across ACT (~127us) and DVE (~108us).

Phases (the span is start ~10.6us + ACT busy + short drain + teardown,
so the schedule is built to keep ACT gapless):
  ramp (blocks 0-14, mode A, per-block DMAs): while the kernel is
      DMA-paced, every block is self-contained on ACT: Exp(x+bias(-e))
      with accum_out -> S, one small DMA per block so ACT never
      burst-waits on a multi-block transfer.  No DVE dependency.
  middle (5 supertiles of 8, ka=2 mode A + 6 mode D): one DMA per
      supertile; mode D does one batched Exp (extra column included,
      exp(-e) via DVE reciprocal) and a DVE fold per block.  Ln of
      supertile k-1 is emitted inside supertile k so ACT never waits
      on the DVE folds; stt (U) of k-1 follows on the DVE.  ka=2 keeps
      the per-supertile DVE load (folds+stt) near the ACT rate so the
      DVE backlog stays under one supertile.
  tail (blocks 55-63, mode A, per-block DMAs): ACT-only dependencies
      again, and the DVE drains its stt backlog here; the final
      supertile is a single block so the Ln+stt drain is ~2.5us.
A dummy 1-element activation at the top hoists the single
ACT_TABLE_LOAD (Exp and Ln share one table set via _PinnedBacc) into
the DMA fill window.
Host: loss = (1/B) * sum over rows/cores of U/S.
"""

import numpy as np

import bass_rust as _bass_rust
import concourse.bacc as bacc
import concourse.bass as bass
import concourse.tile as tile
from concourse import bass_utils, mybir
from concourse.hw_specs import get_activation_tables

N_CORES = 8
B_FULL = 65536
C1 = 1001
C = 1000
P = 128
B_SHARD = B_FULL // N_CORES  # 8192
N_BLOCKS = B_SHARD // P  # 64
G_MAX = 8

# (g, ka, per_block_dma): ka blocks of the supertile are mode A, the rest
# mode D.  per_block_dma issues one DMA per block (ramp/tail smoothing).
PLAN = [
    (1, 1, True), (2, 2, True), (4, 4, True), (8, 8, True),      # ramp: 15 A
    (8, 2, False), (8, 2, False), (8, 2, False), (8, 2, False), (8, 2, False),
    (4, 4, True), (3, 3, True), (1, 1, True), (1, 1, True),      # tail: 9 A
]
assert sum(g for g, _, _ in PLAN) == N_BLOCKS


class _PinnedBacc(bacc.Bacc):
    """Bacc whose activation-table chooser only sees sets containing every
    activation function this kernel uses, so Exp and Ln resolve to one
    resident set (natural_log_exp_and_others) instead of thrashing
    ACT_TABLE_LOADs between per-function sets."""

    def insert_act_table_loads(self):
        used = {
            i.func
            for b in self.main_func.blocks
            for i in b.instructions
            if isinstance(i, mybir.InstActivation)
        }
        if not used:
            return
        tables = [
            (name, fns if used <= fns else set())
            for name, fns in get_activation_tables(self.m.arch).items()
        ]
        _bass_rust.insert_act_table_loads(self, tables)


_nc_cache = None


def _build() -> bass.Bass:
    global _nc_cache
    if _nc_cache is not None:
        return _nc_cache

    nc = _PinnedBacc("TRN2", debug=False)
    x = nc.dram_tensor("x", [B_SHARD, C1], mybir.dt.float32, kind="ExternalInput").ap()
    u_out = nc.dram_tensor(
        "u_out", [P, N_BLOCKS], mybir.dt.float32, kind="ExternalOutput"
    ).ap()
    s_out = nc.dram_tensor(
        "s_out", [P, N_BLOCKS], mybir.dt.float32, kind="ExternalOutput"
    ).ap()

    # Shard row handled by (partition p, block n): row = p*N_BLOCKS + n, so a
    # run of consecutive blocks is contiguous DRAM per partition.
    x_r = x.rearrange("(p n) m -> p n m", p=P, n=N_BLOCKS)

    with tile.TileContext(nc) as tc:
        with (
            tc.tile_pool(name="xin", bufs=3) as xin,
            tc.tile_pool(name="mid", bufs=2) as mid,
            tc.tile_pool(name="small", bufs=3) as small,
            tc.tile_pool(name="accp", bufs=1) as accp,
        ):
            U = accp.tile([P, N_BLOCKS], mybir.dt.float32)
            S = accp.tile([P, N_BLOCKS], mybir.dt.float32)

            # Dummy activation on a constant tile: pulls the ACT_TABLE_LOAD
            # into the DMA fill window instead of after the first data lands.
            warm = accp.tile([P, 2], mybir.dt.float32)
            nc.vector.memset(warm, 0.0)
            nc.scalar.activation(
                out=warm[:, 1:2],
                in_=warm[:, 0:1],
                func=mybir.ActivationFunctionType.Exp,
            )

            n0 = 0  # first block of the current supertile
            prev = None  # (n0, g, aa, ww) of the previous supertile
            for g, ka, per_block in PLAN:
                kd = g - ka
                xt = xin.tile([P, G_MAX, C1], mybir.dt.float32, tag="xt")
                if per_block:
                    for i in range(g):
                        nc.sync.dma_start(
                            out=xt[:, i, :], in_=x_r[:, n0 + i : n0 + i + 1, :]
                        )
                else:
                    nc.sync.dma_start(out=xt[:, 0:g, :], in_=x_r[:, n0 : n0 + g, :])

                tt = None
                if kd:
                    # mode D: one batched Exp over the kd blocks, extra-logit
                    # column included (t[:, :, C] = exp(e)).
                    tt = mid.tile([P, G_MAX, C1], mybir.dt.bfloat16, tag="tt")
                    nc.scalar.activation(
                        out=tt[:, 0:kd, :].rearrange("p g c -> p (g c)"),
                        in_=xt[:, ka:g, :].rearrange("p g c -> p (g c)"),
                        func=mybir.ActivationFunctionType.Exp,
                    )
                    cc = small.tile([P, G_MAX], mybir.dt.float32, tag="cc")
                    nc.vector.reciprocal(cc[:, 0:kd], tt[:, 0:kd, C])

                # ACT: Ln of the previous supertile (pipelined one behind so
                # the DVE folds of k-1 are long done and ACT never stalls).
                if prev is not None:
                    pn0, pg, paa, pww = prev
                    nc.scalar.activation(
                        out=pww[:, 0:pg, :].rearrange("p g c -> p (g c)"),
                        in_=paa[:, 0:pg, :].rearrange("p g c -> p (g c)"),
                        func=mybir.ActivationFunctionType.Ln,
                        bias=1.0,
                        scale=1.0,
                    )

                aa = mid.tile([P, G_MAX, C], mybir.dt.bfloat16, tag="aa")

                # mode A: per-block Exp with bias(-e), accum_out -> S col.
                if ka:
                    neg_e = small.tile([P, G_MAX], mybir.dt.float32, tag="neg_e")
                    for i in range(ka):
                        col = n0 + i
                        nc.vector.tensor_scalar_mul(
                            neg_e[:, i : i + 1], xt[:, i, C : C1], -1.0
                        )
                        nc.scalar.activation(
                            out=aa[:, i, :],
                            in_=xt[:, i, 0:C],
                            func=mybir.ActivationFunctionType.Exp,
                            bias=neg_e[:, i : i + 1],
                            scale=1.0,
                            accum_out=S[:, col : col + 1],
                        )

                # DVE: mode-D fold a = t * exp(-e), accum_out -> S col (1x).
                for j in range(kd):
                    col = n0 + ka + j
                    nc.vector.tensor_scalar(
                        out=aa[:, ka + j, :],
                        in0=tt[:, j, 0:C],
                        scalar1=cc[:, j : j + 1],
                        scalar2=0.0,
                        op0=mybir.AluOpType.mult,
                        op1=mybir.AluOpType.add,
                        accum_out=S[:, col : col + 1],
                    )

                # DVE: U columns of the previous supertile.
                if prev is not None:
                    pn0, pg, paa, pww = prev
                    for i in range(pg):
                        col = pn0 + i
                        scr = mid.tile([P, C], mybir.dt.bfloat16, tag="scr")
                        nc.vector.scalar_tensor_tensor(
                            out=scr,
                            in0=paa[:, i, :],
                            scalar=1.0,
                            in1=pww[:, i, :],
                            op0=mybir.AluOpType.mult,
                            op1=mybir.AluOpType.mult,
                            accum_out=U[:, col : col + 1],
                        )

                ww = mid.tile([P, G_MAX, C], mybir.dt.bfloat16, tag="ww")
                prev = (n0, g, aa, ww)
                n0 += g

            # drain: Ln + U of the last supertile
            pn0, pg, paa, pww = prev
            nc.scalar.activation(
                out=pww[:, 0:pg, :].rearrange("p g c -> p (g c)"),
                in_=paa[:, 0:pg, :].rearrange("p g c -> p (g c)"),
                func=mybir.ActivationFunctionType.Ln,
                bias=1.0,
                scale=1.0,
            )
            for i in range(pg):
                col = pn0 + i
                scr = mid.tile([P, C], mybir.dt.bfloat16, tag="scr")
                nc.vector.scalar_tensor_tensor(
                    out=scr,
                    in0=paa[:, i, :],
                    scalar=1.0,
                    in1=pww[:, i, :],
                    op0=mybir.AluOpType.mult,
                    op1=mybir.AluOpType.mult,
                    accum_out=U[:, col : col + 1],
                )

            nc.sync.dma_start(out=u_out, in_=U)
            nc.sync.dma_start(out=s_out, in_=S)

    nc.finalize()  # runs Bacc passes (wait splitting, reg alloc, ...)
    _nc_cache = nc
    return nc


LAST_RESULTS = None


def kernel(input: np.ndarray, target: np.ndarray | None = None, _trace: bool = False, **_unused) -> np.ndarray:
    global LAST_RESULTS
    input = np.ascontiguousarray(np.asarray(input, dtype=np.float32))
    assert input.shape == (B_FULL, C1), input.shape

    nc = _build()
    in_maps = [
        {"x": input[i * B_SHARD : (i + 1) * B_SHARD]} for i in range(N_CORES)
    ]
    res = bass_utils.run_bass_kernel_spmd(
        nc, in_maps, core_ids=list(range(N_CORES)), trace=_trace
    )
    LAST_RESULTS = res
    total = np.float64(0.0)
    for r in res.results:
        u = np.asarray(r["u_out"], dtype=np.float64)
        s = np.asarray(r["s_out"], dtype=np.float64)
        total += (u / s).sum()
    # w = log1p(a) = -log(pc) already carries the loss's minus sign.
    loss = total / B_FULL
    return np.float32(loss)
